# revision 45
# baseline (speedup 1.0000x reference)
"""GPT forward (L=12, D=1024, H=16, B=2, T=1024, V=32000) on 8 trn2 NeuronCores.

Sharding: balanced sequence-parallel. Core c owns batch c//4 and token
sub-chunks (g, 7-g) of 128 tokens each, g = c%4 — so causal attention work
(9 of 16 kv-tile pairs per head) is identical on every core.

Per layer: LN -> K^T (feature-major) -> AllGather(K) over the 4-core batch
group; V (token-major, 65-stride head layout w/ ones column) -> AllGather(V);
Q while gathers run. Attention: diagonal = LOCAL self-attention of each
sub-chunk (tril mask); interior kv tiles from the gathered K/V with per-core
0/1 kill masks (input data, keeping one SPMD program). Scores/exp batched in
[128,512] tiles; softmax denominator via ones-column in the V matmul; one
batched reciprocal per layer. proj/fc2/V/logits are activation-stationary so
consecutive matmuls share the PE weights. Final LN -> 8-core AllGather
(Shared output, RDH) -> vocab-sharded logits.
"""
import sys
import numpy as np

sys.path.insert(0, '/opt/trn_rl_repo')
import ml_dtypes

BF = ml_dtypes.bfloat16
L, D, H, V, B, T = 12, 1024, 16, 32000, 2, 1024
DH = D // H          # 64
EPS = 1e-5
N_CORES = 8
CHUNK = 256          # tokens per core (2 sub-chunks of 128)
VS = V // N_CORES    # 4000 vocab cols per core
FT = D // 128        # 8 feature tiles
NIT_A = 3            # interior kv tiles for sub-chunk A (tiles 0..2)
NIT_B = 7            # interior kv tiles for sub-chunk B (tiles 0..6)


def host_prep(inputs):
    inputs = {k: np.asarray(v) for k, v in inputs.items()}
    for name in ['ln1_b', 'ln2_b', 'b_qkv', 'b_proj', 'b_fc1', 'b_fc2', 'lnf_b']:
        assert not np.any(inputs[name]), f"{name} nonzero — bias folding unsupported"
    x0 = inputs['wte'][inputs['tokens']] + inputs['wpe'][None, :, :]   # [B,T,D] f32
    w_qkv = inputs['w_qkv'] * inputs['ln1_w'][:, :, None]
    w_fc1 = inputs['w_fc1'] * inputs['ln2_w'][:, :, None]
    w_out = inputs['w_out'] * inputs['lnf_w'][:, None]
    return {
        'x0': np.ascontiguousarray(x0, np.float32),
        'w_qkv': np.ascontiguousarray(w_qkv.astype(BF)),
        'w_proj': np.ascontiguousarray(inputs['w_proj'].astype(BF)),
        'w_fc1': np.ascontiguousarray(w_fc1.astype(BF)),
        'w_fc2': np.ascontiguousarray(inputs['w_fc2'].astype(BF)),
        'w_out': np.ascontiguousarray(w_out.astype(BF)),
    }


def make_masks3(g):
    """[128, 3*512] combined kill/tril masks for core with group index g.

    e1 = [t0A t0B t1A t1B], e2 = [t2A t2B diagA diagB], e3 = [t3B..t6B];
    A-interior tile j valid iff j < g, B-interior tile t valid iff t < 7-g,
    diag blocks get the tril (kv <= q) mask.
    """
    r = np.arange(128)[:, None]
    c = np.arange(128)[None, :]
    tril = (r <= c).astype(np.float32)
    one = np.ones((128, 128), np.float32)
    ka = lambda j: one * (1.0 if j < g else 0.0)
    kb = lambda t: one * (1.0 if t < 7 - g else 0.0)
    m1 = np.concatenate([ka(0), one, ka(1), one], axis=1)
    m2 = np.concatenate([ka(2), one, tril, tril], axis=1)
    m3 = np.concatenate([one, kb(4), kb(5), kb(6)], axis=1)
    return np.concatenate([m1, m2, m3], axis=1).astype(BF)


def make_sel():
    """[97, 256] selectors. Block A (cols 0:128): partition 0 -> rows 0:64,
    partition 32 -> rows 64:128. Block B (cols 128:256): 64 -> 0:64, 96 -> 64:128."""
    s = np.zeros((97, 256), BF)
    s[0, 0:64] = 1
    s[32, 64:128] = 1
    s[64, 128:192] = 1
    s[96, 192:256] = 1
    return s


def build_program(n_layers=L, n_rep=1, bcast_mode='mm', dbg=False):
    import concourse.bass as bass
    import concourse.mybir as mybir
    import concourse.tile as tile
    from concourse import bacc
    from concourse.masks import make_identity
    from contextlib import ExitStack

    f32 = mybir.dt.float32
    bf16 = mybir.dt.bfloat16
    fp8 = mybir.dt.float8e4
    AF = mybir.ActivationFunctionType

    nc = bacc.Bacc('TRN2', target_bir_lowering=False, debug=False, num_devices=N_CORES)

    x0_in = nc.dram_tensor("x0", [CHUNK, D], f32, kind="ExternalInput")
    wq_in = nc.dram_tensor("wq", [n_layers, D, 3 * D], bf16, kind="ExternalInput")
    wp_in = nc.dram_tensor("wp", [n_layers, D, D], bf16, kind="ExternalInput")
    w1_in = nc.dram_tensor("w1", [n_layers, D, D], bf16, kind="ExternalInput")
    w2_in = nc.dram_tensor("w2", [n_layers, D, D], bf16, kind="ExternalInput")
    wo_in = nc.dram_tensor("wo", [D, VS], bf16, kind="ExternalInput")
    msk_in = nc.dram_tensor("masks3", [128, 3 * 512], bf16, kind="ExternalInput")
    sel_in = nc.dram_tensor("sel", [97, 256], bf16, kind="ExternalInput")
    out_ext = nc.dram_tensor("logits", [N_CORES * CHUNK, VS], f32, kind="ExternalOutput")
    dbg_ext = nc.dram_tensor("dbg", [128, 4096], bf16, kind="ExternalOutput") if dbg else None
    dbgf_ext = nc.dram_tensor("dbgf", [128, 256], f32, kind="ExternalOutput") if dbg else None

    # per (rep, layer) collective buffers
    k_locs, k_alls, v_locs, v_alls, xf_locs, xf_alls = [], [], [], [], [], []
    for r in range(n_rep):
        k_locs.append([nc.dram_tensor(f"kl_{r}_{l}", [D, CHUNK], bf16)
                       for l in range(n_layers)])
        k_alls.append([nc.dram_tensor(f"ka_{r}_{l}", [4 * D, CHUNK], bf16)
                       for l in range(n_layers)])
        v_locs.append([nc.dram_tensor(f"vl_{r}_{l}", [CHUNK, D], bf16)
                       for l in range(n_layers)])
        v_alls.append([nc.dram_tensor(f"va_{r}_{l}", [4 * CHUNK, D], bf16)
                       for l in range(n_layers)])
        xf_locs.append(nc.dram_tensor(f"xfl_{r}", [D, CHUNK], bf16))
        xf_alls.append(nc.dram_tensor(f"xfa_{r}", [N_CORES * D, CHUNK], bf16,
                                      addr_space="Shared"))
    prime_l = nc.dram_tensor("prml", [1, 64], bf16)
    prime_a = nc.dram_tensor("prma", [1, 256], bf16)

    groups_b = [[0, 1, 2, 3], [4, 5, 6, 7]]
    group_all = [list(range(N_CORES))]

    def dram_ap(handle, offset, ap):
        base = handle[:, :]
        return bass.AP(tensor=base.tensor, offset=offset, ap=ap)

    def _patch_tile_name(pool):
        orig = pool.tile
        def tile_(shape, dtype, *, tag="", **kw):
            kw.setdefault("name", tag or "t")
            return orig(shape, dtype, tag=tag, **kw)
        pool.tile = tile_
        return pool

    with tile.TileContext(nc) as tc, ExitStack() as ctx:
        persist = _patch_tile_name(ctx.enter_context(tc.tile_pool(name="persist", bufs=1)))
        x_t = [persist.tile([128, D], f32, tag=f"x{m}") for m in range(2)]
        for m in range(2):
            nc.sync.dma_start(out=x_t[m], in_=x0_in[128 * m:128 * (m + 1), :])
        ident = persist.tile([128, 128], bf16, tag="ident")
        make_identity(nc, ident)
        eps_t = persist.tile([128, 1], f32, tag="eps")
        nc.vector.memset(eps_t, EPS)
        msk_t = persist.tile([128, 3 * 512], bf16, tag="masks3")
        nc.sync.dma_start(out=msk_t, in_=msk_in[:, :])
        m1, m2, m3 = (msk_t[:, 512 * i:512 * (i + 1)] for i in range(3))
        prm = persist.tile([1, 64], bf16, tag="prm")
        nc.vector.memset(prm, 0.0)
        # selector matrices (heads grouped 4 per den tile at partitions
        # 0/32/64/96); sel block A covers even ft, block B odd ft
        sel_t = persist.tile([97, 256], bf16, tag="sel")
        nc.sync.dma_start(out=sel_t, in_=sel_in[:, :])
        # denominator collection tiles: 4 heads each, rows at 0/32/64/96;
        # other partitions memset to 1 so reciprocal stays finite
        den4 = [persist.tile([97, CHUNK], f32, tag=f"den4_{i}") for i in range(4)]
        den4r = [persist.tile([97, CHUNK], bf16, tag=f"den4r{i}") for i in range(4)]
        for i in range(4):
            nc.vector.memset(den4[i], 1.0)

        ln_pool = _patch_tile_name(ctx.enter_context(tc.tile_pool(name="ln", bufs=1)))
        wpool = _patch_tile_name(ctx.enter_context(tc.tile_pool(name="weights", bufs=1)))
        apool = _patch_tile_name(ctx.enter_context(tc.tile_pool(name="acts", bufs=1)))
        kvpool = _patch_tile_name(ctx.enter_context(tc.tile_pool(name="kv", bufs=1)))
        spool = _patch_tile_name(ctx.enter_context(tc.tile_pool(name="small", bufs=2)))

        def layernorm_T(xt, ps_a):
            """LN both token tiles of x -> feature-major bf16 tiles [128, 256] x8."""
            xh = []
            for m in range(2):
                stats = spool.tile([128, 2, 6], f32, tag="lnstats")
                nc.vector.bn_stats(out=stats[:, 0, :], in_=xt[m][:, 0:512])
                nc.vector.bn_stats(out=stats[:, 1, :], in_=xt[m][:, 512:1024])
                mv = spool.tile([128, 2], f32, tag="lnmv")
                nc.vector.bn_aggr(out=mv, in_=stats)
                rs = spool.tile([128, 1], f32, tag="lnrs")
                nc.scalar.activation(out=rs, in_=mv[:, 1:2], func=AF.Sqrt, bias=eps_t)
                nc.vector.reciprocal(out=rs, in_=rs)
                xh_m = ln_pool.tile([128, D], bf16, tag=f"lnxh{m}")
                nc.vector.tensor_scalar(
                    out=xh_m, in0=xt[m], scalar1=mv[:, 0:1], scalar2=rs,
                    op0=mybir.AluOpType.subtract, op1=mybir.AluOpType.mult)
                xh.append(xh_m)
            xhT = [ln_pool.tile([128, CHUNK], bf16, tag=f"lnxhT{t}") for t in range(FT)]
            for t in range(FT):
                for m in range(2):
                    ptr = ps_a.tile([128, 128], bf16, tag="ps256", padded_shape=[128, 512])
                    nc.tensor.transpose(ptr, xh[m][:, 128 * t:128 * (t + 1)], ident)
                    nc.vector.tensor_copy(out=xhT[t][:, 128 * m:128 * (m + 1)], in_=ptr)
            return xhT

        with ExitStack() as lctx:
            ps_a = _patch_tile_name(lctx.enter_context(
                tc.tile_pool(name="ps_a", bufs=2, space="PSUM")))
            ps_b = _patch_tile_name(lctx.enter_context(
                tc.tile_pool(name="ps_b", bufs=2, space="PSUM")))
            ps_att = _patch_tile_name(lctx.enter_context(
                tc.tile_pool(name="ps_att", bufs=2, space="PSUM")))

            # prime the collective stack: tiny AG absorbs the initial barrier
            nc.gpsimd.collective_compute(
                "AllGather", mybir.AluOpType.bypass, replica_groups=groups_b,
                ins=[prime_l[:, :]], outs=[prime_a[:, :]])

            for rep in range(n_rep):
                for l in range(n_layers):
                    _fsid, _ = nc.enter_named_scope(f"L{l}.layer", False)
                    xhT = layernorm_T(x_t, ps_a)
                    k_loc, k_all = k_locs[rep][l], k_alls[rep][l]
                    v_loc, v_all = v_locs[rep][l], v_alls[rep][l]

                    # ---- K^T (feature-major) + ship + AllGather ----
                    wk_t = [wpool.tile([128, D], bf16, tag=f"wk{kk}") for kk in range(FT)]
                    for kk in range(FT):
                        nc.sync.dma_start(
                            out=wk_t[kk],
                            in_=wq_in[l, 128 * kk:128 * (kk + 1), D:2 * D])
                    kTl = [apool.tile([128, CHUNK], bf16, tag=f"kT{t}") for t in range(FT)]
                    for f in range(FT):
                        ps = ps_a.tile([128, CHUNK], f32, tag="ps256")
                        for kk in range(FT):
                            nc.tensor.matmul(ps, wk_t[kk][:, 128 * f:128 * (f + 1)],
                                             xhT[kk], start=(kk == 0), stop=(kk == FT - 1))
                        if f % 2 == 0:
                            nc.vector.tensor_copy(out=kTl[f], in_=ps)
                        else:
                            nc.scalar.activation(out=kTl[f], in_=ps, func=AF.Copy)
                        nc.sync.dma_start(
                            out=dram_ap(k_loc, 128 * f * CHUNK, [[CHUNK, 128], [1, CHUNK]]),
                            in_=kTl[f])
                    nc.gpsimd.collective_compute(
                        "AllGather", mybir.AluOpType.bypass, replica_groups=groups_b,
                        ins=[k_loc[:, :]], outs=[k_all[:, :]])

                    # ---- V (token-major, 65-stride head layout) + ship + AllGather ----
                    wv_t = [wpool.tile([128, D], bf16, tag=f"wv{kk}") for kk in range(FT)]
                    for kk in range(FT):
                        nc.sync.dma_start(
                            out=wv_t[kk],
                            in_=wq_in[l, 128 * kk:128 * (kk + 1), 2 * D:3 * D])
                    v_t = [kvpool.tile([128, 16 * 65], bf16, tag=f"vloc{m}") for m in range(2)]
                    for m in range(2):
                        psA = ps_b.tile([128, 512], f32, tag="ps512")
                        psB = ps_b.tile([128, 512], f32, tag="ps512")
                        for kk in range(FT):
                            lhs = xhT[kk][:, 128 * m:128 * (m + 1)]
                            nc.tensor.matmul(psA, lhs, wv_t[kk][:, 0:512],
                                             start=(kk == 0), stop=(kk == FT - 1))
                            nc.tensor.matmul(psB, lhs, wv_t[kk][:, 512:1024],
                                             start=(kk == 0), stop=(kk == FT - 1))
                        ve = v_t[m].rearrange("p (h c) -> p h c", h=16)
                        nc.vector.tensor_copy(out=ve[:, 0:8, 0:64], in_=psA)
                        nc.vector.tensor_copy(out=ve[:, 8:16, 0:64], in_=psB)
                        nc.vector.memset(ve[:, :, 64:65], 1.0)
                        nc.sync.dma_start(
                            out=dram_ap(v_loc, 128 * m * D, [[D, 128], [1, D]]),
                            in_=ve[:, :, 0:64])
                    nc.gpsimd.collective_compute(
                        "AllGather", mybir.AluOpType.bypass, replica_groups=groups_b,
                        ins=[v_loc[:, :]], outs=[v_all[:, :]])

                    # ---- Q (feature-major) ----
                    wqq_t = [wpool.tile([128, D], bf16, tag=f"wqq{kk}") for kk in range(FT)]
                    for kk in range(FT):
                        nc.sync.dma_start(
                            out=wqq_t[kk],
                            in_=wq_in[l, 128 * kk:128 * (kk + 1), 0:D])
                    qT = [apool.tile([128, CHUNK], bf16, tag=f"qT{t}") for t in range(FT)]
                    for f in range(FT):
                        ps = ps_a.tile([128, CHUNK], f32, tag="ps256")
                        for kk in range(FT):
                            nc.tensor.matmul(ps, wqq_t[kk][:, 128 * f:128 * (f + 1)],
                                             xhT[kk], start=(kk == 0), stop=(kk == FT - 1))
                        if f % 2 == 0:
                            nc.vector.tensor_copy(out=qT[f], in_=ps)
                        else:
                            nc.scalar.activation(out=qT[f], in_=ps, func=AF.Copy)

                    # ---- load gathered interior K tiles (t = 0..6) ----
                    # absolute kv tile t: t<4 -> rank t slot A; t>=4 -> rank 7-t slot B
                    kall = [kvpool.tile([128, 7 * 128], bf16, tag=f"kft{ft}")
                            for ft in range(FT)]
                    kcol = lambda t: 128 * t if t < 4 else 128 * (10 - t)
                    for ft in range(FT):
                        nc.sync.dma_start(
                            out=kall[ft][:, 0:512].rearrange("p (r c) -> p r c", r=4),
                            in_=dram_ap(k_all, 128 * ft * CHUNK,
                                        [[CHUNK, 128], [D * CHUNK, 4], [1, 128]]))
                        nc.sync.dma_start(
                            out=kall[ft][:, 512:896].rearrange("p (r c) -> p r c", r=3),
                            in_=dram_ap(k_all, D * CHUNK + 128 * ft * CHUNK + 128,
                                        [[CHUNK, 128], [D * CHUNK, 3], [1, 128]]))
                    # gathered interior V tiles in 65-stride layout + ones col
                    vext = [kvpool.tile([128, 16 * 65], bf16, tag=f"vext{t}")
                            for t in range(NIT_B)]
                    for t in range(NIT_B):
                        rk, sl = (t, 0) if t < 4 else (7 - t, 1)
                        ve = vext[t].rearrange("p (h c) -> p h c", h=16)
                        nc.sync.dma_start(
                            out=ve[:, :, 0:64],
                            in_=dram_ap(v_all, (rk * CHUNK + 128 * sl) * D,
                                        [[D, 128], [64, 16], [1, 64]]))
                        nc.vector.memset(ve[:, :, 64:65], 1.0)

                    # ---- attention ----
                    attnN = [apool.tile([128, CHUNK], bf16, tag=f"attnN{t}")
                             for t in range(FT)]
                    dbg_e = []
                    for h in range(H):
                        ft, ro = h // 2, 64 * (h % 2)
                        kT_h = lambda t: kall[ft][ro:ro + 64, kcol(t):kcol(t) + 128]
                        qA = qT[ft][ro:ro + 64, 0:128]
                        qB = qT[ft][ro:ro + 64, 128:256]
                        # scores: S1 = [t0 | t1], S2 = [t2 | diagA | diagB], S3 = [t3..t6]
                        s1 = ps_b.tile([128, 512], f32, tag="ps512")
                        nc.tensor.matmul(s1[:, 0:256], kT_h(0), qT[ft][ro:ro + 64, :],
                                         start=True, stop=True)
                        nc.tensor.matmul(s1[:, 256:512], kT_h(1), qT[ft][ro:ro + 64, :],
                                         start=True, stop=True)
                        s2 = ps_b.tile([128, 512], f32, tag="ps512")
                        nc.tensor.matmul(s2[:, 0:256], kT_h(2), qT[ft][ro:ro + 64, :],
                                         start=True, stop=True)
                        nc.tensor.matmul(s2[:, 256:384],
                                         kTl[ft][ro:ro + 64, 0:128], qA,
                                         start=True, stop=True)
                        nc.tensor.matmul(s2[:, 384:512],
                                         kTl[ft][ro:ro + 64, 128:256], qB,
                                         start=True, stop=True)
                        s3 = ps_b.tile([128, 512], f32, tag="ps512")
                        for t in range(3, NIT_B):
                            nc.tensor.matmul(s3[:, 128 * (t - 3):128 * (t - 2)],
                                             kT_h(t), qB, start=True, stop=True)
                        e1 = spool.tile([128, 512], bf16, tag="e1", bufs=3)
                        e2 = spool.tile([128, 512], bf16, tag="e2", bufs=3)
                        e3 = spool.tile([128, 512], bf16, tag="e3", bufs=3)
                        nc.scalar.activation(out=e1, in_=s1, func=AF.Exp, scale=0.125)
                        nc.scalar.activation(out=e2, in_=s2, func=AF.Exp, scale=0.125)
                        nc.scalar.activation(out=e3, in_=s3, func=AF.Exp, scale=0.125)
                        # combined per-core kill + tril masks (one TT per e tile)
                        nc.vector.tensor_mul(out=e1, in0=e1, in1=m1)
                        nc.vector.tensor_mul(out=e2, in0=e2, in1=m2)
                        nc.vector.tensor_mul(out=e3, in0=e3, in1=m3)
                        if dbg and l == 0 and h == 0:
                            dbg_e = [e1, e2]
                        # av accumulation: regions in SEPARATE psum banks —
                        # interleaved accumulating groups sharing a bank break
                        # (a start=True zeroes the other group's partial sums)
                        attA = ps_att.tile([65, 128], f32, tag="attA")
                        attB = ps_att.tile([65, 128], f32, tag="attB")
                        vh = lambda vt: vt[:, 65 * h:65 * h + 65]
                        nc.tensor.matmul(attA, vh(v_t[0]), e2[:, 256:384],
                                         start=True, stop=False)
                        nc.tensor.matmul(attB, vh(v_t[1]), e2[:, 384:512],
                                         start=True, stop=False)
                        rhsA = [e1[:, 0:128], e1[:, 256:384], e2[:, 0:128]]
                        rhsB = [e1[:, 128:256], e1[:, 384:512], e2[:, 128:256]]
                        for t in range(NIT_A):
                            nc.tensor.matmul(attA, vh(vext[t]), rhsA[t],
                                             start=False, stop=(t == NIT_A - 1))
                            nc.tensor.matmul(attB, vh(vext[t]), rhsB[t],
                                             start=False, stop=False)
                        for t in range(3, NIT_B):
                            nc.tensor.matmul(attB, vh(vext[t]),
                                             e3[:, 128 * (t - 3):128 * (t - 2)],
                                             start=False, stop=(t == NIT_B - 1))
                        p0 = 32 * (h % 4)
                        nc.vector.tensor_copy(out=den4[h // 4][p0:p0 + 1, 0:128],
                                              in_=attA[64:65, :])
                        nc.vector.tensor_copy(out=den4[h // 4][p0:p0 + 1, 128:256],
                                              in_=attB[64:65, :])
                        nc.scalar.activation(out=attnN[ft][ro:ro + 64, 0:128],
                                             in_=attA[0:64, :], func=AF.Copy)
                        nc.scalar.activation(out=attnN[ft][ro:ro + 64, 128:256],
                                             in_=attB[0:64, :], func=AF.Copy)

                    for i in range(4):
                        dr = spool.tile([97, CHUNK], f32, tag="denr", bufs=2)
                        nc.vector.reciprocal(out=dr, in_=den4[i])
                        nc.vector.tensor_copy(out=den4r[i], in_=dr)
                    attnT = [apool.tile([128, CHUNK], bf16, tag=f"kT{t}")
                             for t in range(FT)]
                    for ft in range(FT):
                        rb = ps_a.tile([128, CHUNK], f32, tag="ps256")
                        lhs = sel_t[:, 0:128] if ft % 2 == 0 else sel_t[:, 128:256]
                        nc.tensor.matmul(rb, lhs, den4r[ft // 2], start=True, stop=True)
                        nc.vector.tensor_mul(out=attnT[ft], in0=attnN[ft], in1=rb)

                    if dbg and l == 0:
                        def dump(col, ap, rows=128):
                            nc.sync.dma_start(
                                out=dbg_ext[0:rows, col:col + ap.shape[-1]], in_=ap)
                        dump(0, v_t[0][:, 0:1040])        # local V ve layout
                        dump(1040, kTl[0][:, :])          # local K^T ft=0
                        dump(1296, qT[0][:, :])           # Q ft=0
                        dump(1808, attnN[0][:, :])        # numerators ft=0
                        dump(2064, attnT[0][:, :])        # normalized ft=0
                        dump(2320, kall[0][:, :])         # gathered K ft=0
                        dump(3216, dbg_e[0][:, :])        # e1 of head 0
                        dump(3728, dbg_e[1][:, 0:368])    # e2 of head 0 (truncated)
                        nc.sync.dma_start(out=dbgf_ext[0:97, :],
                                          in_=den4[0][:, :])   # denominators heads 0-3

                    # ---- proj + residual (activation-stationary) ----
                    wp_t = [wpool.tile([128, D], bf16, tag=f"wp{kk}") for kk in range(FT)]
                    for kk in range(FT):
                        nc.sync.dma_start(out=wp_t[kk],
                                          in_=wp_in[l, 128 * kk:128 * (kk + 1), :])
                    for m in range(2):
                        psA = ps_b.tile([128, 512], f32, tag="ps512")
                        psB = ps_b.tile([128, 512], f32, tag="ps512")
                        for kk in range(FT):
                            lhs = attnT[kk][:, 128 * m:128 * (m + 1)]
                            nc.tensor.matmul(psA, lhs, wp_t[kk][:, 0:512],
                                             start=(kk == 0), stop=(kk == FT - 1))
                            nc.tensor.matmul(psB, lhs, wp_t[kk][:, 512:1024],
                                             start=(kk == 0), stop=(kk == FT - 1))
                        nc.vector.tensor_add(out=x_t[m][:, 0:512],
                                             in0=x_t[m][:, 0:512], in1=psA)
                        nc.vector.tensor_add(out=x_t[m][:, 512:1024],
                                             in0=x_t[m][:, 512:1024], in1=psB)

                    # ---- MLP ----
                    hT = layernorm_T(x_t, ps_a)
                    w1_t = [wpool.tile([128, D], bf16, tag=f"w1{kk}", bufs=2)
                            for kk in range(FT)]
                    w2_t = [wpool.tile([128, D], bf16, tag=f"w2{kk}")
                            for kk in range(FT)]
                    for kk in range(FT):
                        nc.sync.dma_start(out=w1_t[kk],
                                          in_=w1_in[l, 128 * kk:128 * (kk + 1), :])
                        nc.sync.dma_start(out=w2_t[kk],
                                          in_=w2_in[l, 128 * kk:128 * (kk + 1), :])
                    gT = [apool.tile([128, CHUNK], bf16, tag=f"qT{t}") for t in range(FT)]
                    for f in range(FT):
                        ps = ps_a.tile([128, CHUNK], f32, tag="ps256")
                        for kk in range(FT):
                            nc.tensor.matmul(ps, w1_t[kk][:, 128 * f:128 * (f + 1)],
                                             hT[kk], start=(kk == 0), stop=(kk == FT - 1))
                        nc.scalar.activation(out=gT[f], in_=ps, func=AF.Gelu)
                    for m in range(2):
                        psA = ps_b.tile([128, 512], f32, tag="ps512")
                        psB = ps_b.tile([128, 512], f32, tag="ps512")
                        for kk in range(FT):
                            lhs = gT[kk][:, 128 * m:128 * (m + 1)]
                            nc.tensor.matmul(psA, lhs, w2_t[kk][:, 0:512],
                                             start=(kk == 0), stop=(kk == FT - 1))
                            nc.tensor.matmul(psB, lhs, w2_t[kk][:, 512:1024],
                                             start=(kk == 0), stop=(kk == FT - 1))
                        nc.vector.tensor_add(out=x_t[m][:, 0:512],
                                             in0=x_t[m][:, 0:512], in1=psA)
                        nc.vector.tensor_add(out=x_t[m][:, 512:1024],
                                             in0=x_t[m][:, 512:1024], in1=psB)
                    nc.leave_named_scope(f"L{l}.layer", _fsid, False)

                # ---- final LN + AllGather ----
                _fsid, _ = nc.enter_named_scope("fin.ag", False)
                xfT = layernorm_T(x_t, ps_a)
                xf_loc, xf_all = xf_locs[rep], xf_alls[rep]
                for t in range(FT):
                    nc.sync.dma_start(
                        out=dram_ap(xf_loc, 128 * t * CHUNK, [[CHUNK, 128], [1, CHUNK]]),
                        in_=xfT[t])
                nc.gpsimd.collective_compute(
                    "AllGather", mybir.AluOpType.bypass, replica_groups=group_all,
                    ins=[xf_loc[:, :]], outs=[xf_all[:, :]])
                nc.leave_named_scope("fin.ag", _fsid, False)

        # ---- logits (activation-stationary over vocab chunks) ----
        with ExitStack() as gctx:
            ps_lg = _patch_tile_name(gctx.enter_context(
                tc.tile_pool(name="ps_lg", bufs=4, space="PSUM")))
            _fsid, _ = nc.enter_named_scope("fin.logits", False)
            NCH = [1024, 1024, 1024, 928]
            for rep in range(n_rep):
                xf_all = xf_alls[rep]
                for mh in range(2):   # halves of the 2048 gathered tokens
                    # vocab weights for the first chunk don't depend on the
                    # AllGather — issue them before the AG-blocked xall loads
                    # so they stream during the collective
                    won0 = None
                    if mh == 0:
                        won0 = [wpool.tile([128, 1024], bf16, tag=f"w1{kk}", bufs=2)
                                for kk in range(FT)]
                        for kk in range(FT):
                            nc.sync.dma_start(
                                out=won0[kk][:, :NCH[0]],
                                in_=wo_in[128 * kk:128 * (kk + 1), 0:NCH[0]])
                    xall = [wpool.tile([128, 1024], bf16, tag=f"wk{kk}")
                            for kk in range(FT)]
                    for kk in range(FT):
                        nc.sync.dma_start(
                            out=xall[kk].rearrange("p (r c) -> p r c", r=4),
                            in_=dram_ap(
                                xf_all, (4 * mh) * D * CHUNK + 128 * kk * CHUNK,
                                [[CHUNK, 128], [D * CHUNK, 4], [1, CHUNK]]))
                    for n in range(4):
                        n0 = 1024 * n
                        if n == 0 and won0 is not None:
                            won = won0
                        else:
                            won = [wpool.tile([128, 1024], bf16, tag=f"w1{kk}", bufs=2)
                                   for kk in range(FT)]
                            for kk in range(FT):
                                nc.sync.dma_start(
                                    out=won[kk][:, :NCH[n]],
                                    in_=wo_in[128 * kk:128 * (kk + 1), n0:n0 + NCH[n]])
                        for mm in range(8):
                            ps = ps_lg.tile([128, 1024], f32, tag="lg")
                            for kk in range(FT):
                                lhs = xall[kk][:, 128 * mm:128 * (mm + 1)]
                                nc.tensor.matmul(ps[:, 0:512], lhs, won[kk][:, 0:512],
                                                 start=(kk == 0), stop=(kk == FT - 1))
                                nc.tensor.matmul(ps[:, 512:NCH[n]], lhs,
                                                 won[kk][:, 512:NCH[n]],
                                                 start=(kk == 0), stop=(kk == FT - 1))
                            lg = spool.tile([128, 1024], f32, tag="lgout", bufs=2)
                            if mm % 2 == 0:
                                nc.vector.tensor_copy(out=lg[:, :NCH[n]], in_=ps[:, :NCH[n]])
                            else:
                                nc.scalar.activation(out=lg[:, :NCH[n]], in_=ps[:, :NCH[n]],
                                                     func=AF.Copy)
                            row0 = (8 * mh + mm) * 128
                            nc.sync.dma_start(
                                out=out_ext[row0:row0 + 128, n0:n0 + NCH[n]],
                                in_=lg[:, :NCH[n]])
            nc.leave_named_scope("fin.logits", _fsid, False)

    nc.compile()
    return nc


_CACHE = {}


def _get_program(n_layers=L, n_rep=1, bcast_mode='mm'):
    key = (n_layers, n_rep, bcast_mode)
    if key not in _CACHE:
        _CACHE[key] = build_program(n_layers, n_rep, bcast_mode)
    return _CACHE[key]


_LAST_RESULT = None


def run_model(prep, n_layers=L, n_rep=1, bcast_mode='mm', **run_kwargs):
    global _LAST_RESULT
    from concourse.bass_utils import run_bass_kernel_spmd
    nc = _get_program(n_layers, n_rep, bcast_mode)
    sel = make_sel()
    in_maps = []
    for c in range(N_CORES):
        b, g = c // 4, c % 4
        rows = np.concatenate([
            prep['x0'][b, 128 * g:128 * (g + 1), :],
            prep['x0'][b, 128 * (7 - g):128 * (8 - g), :]])
        in_maps.append({
            'x0': np.ascontiguousarray(rows),
            'wq': prep['w_qkv'][:n_layers],
            'wp': prep['w_proj'][:n_layers],
            'w1': prep['w_fc1'][:n_layers],
            'w2': prep['w_fc2'][:n_layers],
            'wo': np.ascontiguousarray(prep['w_out'][:, VS * c:VS * (c + 1)]),
            'masks3': make_masks3(g),
            'sel': sel,
        })
    res = run_bass_kernel_spmd(nc, in_maps, core_ids=list(range(N_CORES)), **run_kwargs)
    _LAST_RESULT = res
    parts = [res.results[c]['logits'] for c in range(N_CORES)]   # [2048, 4000] each
    wide = np.concatenate(parts, axis=1)                          # [2048, 32000] permuted rows
    # invert the row permutation: rank r holds sub-chunks (g, 7-g) of batch r//4
    full = np.empty((B, T, V), np.float32)
    for r in range(N_CORES):
        b, g = r // 4, r % 4
        full[b, 128 * g:128 * (g + 1)] = wide[256 * r:256 * r + 128]
        full[b, 128 * (7 - g):128 * (8 - g)] = wide[256 * r + 128:256 * (r + 1)]
    return full


def kernel(**inputs):
    prep = host_prep(inputs)
    return run_model(prep)


# revision 47
# speedup vs baseline: 1.0305x; 1.0305x over previous
"""GPT forward (L=12, D=1024, H=16, B=2, T=1024, V=32000) on 8 trn2 NeuronCores.

Sharding: balanced sequence-parallel. Core c owns batch c//4 and token
sub-chunks (g, 7-g) of 128 tokens each, g = c%4 — so causal attention work
(9 of 16 kv-tile pairs per head) is identical on every core.

Per layer: LN -> K^T (feature-major) -> AllGather(K) over the 4-core batch
group; V (token-major, 65-stride head layout w/ ones column) -> AllGather(V);
Q while gathers run. Attention: diagonal = LOCAL self-attention of each
sub-chunk (tril mask); interior kv tiles from the gathered K/V with per-core
0/1 kill masks (input data, keeping one SPMD program). Scores/exp batched in
[128,512] tiles; softmax denominator via ones-column in the V matmul; one
batched reciprocal per layer. proj/fc2/V/logits are activation-stationary so
consecutive matmuls share the PE weights. Final LN -> 8-core AllGather
(Shared output, RDH) -> vocab-sharded logits.
"""
import sys
import numpy as np

sys.path.insert(0, '/opt/trn_rl_repo')
import ml_dtypes

BF = ml_dtypes.bfloat16
L, D, H, V, B, T = 12, 1024, 16, 32000, 2, 1024
DH = D // H          # 64
EPS = 1e-5
N_CORES = 8
CHUNK = 256          # tokens per core (2 sub-chunks of 128)
VS = V // N_CORES    # 4000 vocab cols per core
FT = D // 128        # 8 feature tiles
NIT_A = 3            # interior kv tiles for sub-chunk A (tiles 0..2)
NIT_B = 7            # interior kv tiles for sub-chunk B (tiles 0..6)


def host_prep(inputs):
    inputs = {k: np.asarray(v) for k, v in inputs.items()}
    for name in ['ln1_b', 'ln2_b', 'b_qkv', 'b_proj', 'b_fc1', 'b_fc2', 'lnf_b']:
        assert not np.any(inputs[name]), f"{name} nonzero — bias folding unsupported"
    x0 = inputs['wte'][inputs['tokens']] + inputs['wpe'][None, :, :]   # [B,T,D] f32
    w_qkv = inputs['w_qkv'] * inputs['ln1_w'][:, :, None]
    w_fc1 = inputs['w_fc1'] * inputs['ln2_w'][:, :, None]
    w_out = inputs['w_out'] * inputs['lnf_w'][:, None]
    return {
        'x0': np.ascontiguousarray(x0, np.float32),
        'w_qkv': np.ascontiguousarray(w_qkv.astype(BF)),
        'w_proj': np.ascontiguousarray(inputs['w_proj'].astype(BF)),
        'w_fc1': np.ascontiguousarray(w_fc1.astype(BF)),
        'w_fc2': np.ascontiguousarray(inputs['w_fc2'].astype(BF)),
        'w_out': np.ascontiguousarray(w_out.astype(BF)),
    }


def make_masks3(g):
    """[128, 3*512] combined kill/tril masks for core with group index g.

    e1 = [t0A t0B t1A t1B], e2 = [t2A t2B diagA diagB], e3 = [t3B..t6B];
    A-interior tile j valid iff j < g, B-interior tile t valid iff t < 7-g,
    diag blocks get the tril (kv <= q) mask.
    """
    r = np.arange(128)[:, None]
    c = np.arange(128)[None, :]
    tril = (r <= c).astype(np.float32)
    one = np.ones((128, 128), np.float32)
    ka = lambda j: one * (1.0 if j < g else 0.0)
    kb = lambda t: one * (1.0 if t < 7 - g else 0.0)
    m1 = np.concatenate([ka(0), one, ka(1), one], axis=1)
    m2 = np.concatenate([ka(2), one, tril, tril], axis=1)
    m3 = np.concatenate([one, kb(4), kb(5), kb(6)], axis=1)
    return np.concatenate([m1, m2, m3], axis=1).astype(BF)


def make_sel():
    """[97, 256] selectors. Block A (cols 0:128): partition 0 -> rows 0:64,
    partition 32 -> rows 64:128. Block B (cols 128:256): 64 -> 0:64, 96 -> 64:128."""
    s = np.zeros((97, 256), BF)
    s[0, 0:64] = 1
    s[32, 64:128] = 1
    s[64, 128:192] = 1
    s[96, 192:256] = 1
    return s


def build_program(n_layers=L, n_rep=1, bcast_mode='mm', dbg=False):
    import concourse.bass as bass
    import concourse.mybir as mybir
    import concourse.tile as tile
    from concourse import bacc
    from concourse.masks import make_identity
    from contextlib import ExitStack

    f32 = mybir.dt.float32
    bf16 = mybir.dt.bfloat16
    fp8 = mybir.dt.float8e4
    AF = mybir.ActivationFunctionType

    nc = bacc.Bacc('TRN2', target_bir_lowering=False, debug=False, num_devices=N_CORES)

    x0_in = nc.dram_tensor("x0", [CHUNK, D], f32, kind="ExternalInput")
    wq_in = nc.dram_tensor("wq", [n_layers, D, 3 * D], bf16, kind="ExternalInput")
    wp_in = nc.dram_tensor("wp", [n_layers, D, D], bf16, kind="ExternalInput")
    w1_in = nc.dram_tensor("w1", [n_layers, D, D], bf16, kind="ExternalInput")
    w2_in = nc.dram_tensor("w2", [n_layers, D, D], bf16, kind="ExternalInput")
    wo_in = nc.dram_tensor("wo", [D, VS], bf16, kind="ExternalInput")
    msk_in = nc.dram_tensor("masks3", [128, 3 * 512], bf16, kind="ExternalInput")
    sel_in = nc.dram_tensor("sel", [97, 256], bf16, kind="ExternalInput")
    out_ext = nc.dram_tensor("logits", [N_CORES * CHUNK, VS], f32, kind="ExternalOutput")
    dbg_ext = nc.dram_tensor("dbg", [128, 4096], bf16, kind="ExternalOutput") if dbg else None
    dbgf_ext = nc.dram_tensor("dbgf", [128, 256], f32, kind="ExternalOutput") if dbg else None

    # per (rep, layer) collective buffers
    k_locs, k_alls, v_locs, v_alls, xf_locs, xf_alls = [], [], [], [], [], []
    for r in range(n_rep):
        k_locs.append([nc.dram_tensor(f"kl_{r}_{l}", [D, CHUNK], bf16)
                       for l in range(n_layers)])
        k_alls.append([nc.dram_tensor(f"ka_{r}_{l}", [4 * D, CHUNK], bf16)
                       for l in range(n_layers)])
        v_locs.append([nc.dram_tensor(f"vl_{r}_{l}", [CHUNK, D], bf16)
                       for l in range(n_layers)])
        v_alls.append([nc.dram_tensor(f"va_{r}_{l}", [4 * CHUNK, D], bf16)
                       for l in range(n_layers)])
        xf_locs.append(nc.dram_tensor(f"xfl_{r}", [D, CHUNK], bf16))
        xf_alls.append(nc.dram_tensor(f"xfa_{r}", [N_CORES * D, CHUNK], bf16,
                                      addr_space="Shared"))
    prime_l = nc.dram_tensor("prml", [1, 64], bf16)
    prime_a = nc.dram_tensor("prma", [1, 256], bf16)

    groups_b = [[0, 1, 2, 3], [4, 5, 6, 7]]
    group_all = [list(range(N_CORES))]

    def dram_ap(handle, offset, ap):
        base = handle[:, :]
        return bass.AP(tensor=base.tensor, offset=offset, ap=ap)

    def _patch_tile_name(pool):
        orig = pool.tile
        def tile_(shape, dtype, *, tag="", **kw):
            kw.setdefault("name", tag or "t")
            return orig(shape, dtype, tag=tag, **kw)
        pool.tile = tile_
        return pool

    with tile.TileContext(nc) as tc, ExitStack() as ctx:
        persist = _patch_tile_name(ctx.enter_context(tc.tile_pool(name="persist", bufs=1)))
        x_t = [persist.tile([128, D], f32, tag=f"x{m}") for m in range(2)]
        for m in range(2):
            nc.sync.dma_start(out=x_t[m], in_=x0_in[128 * m:128 * (m + 1), :])
        ident = persist.tile([128, 128], bf16, tag="ident")
        make_identity(nc, ident)
        eps_t = persist.tile([128, 1], f32, tag="eps")
        nc.vector.memset(eps_t, EPS)
        msk_t = persist.tile([128, 3 * 512], bf16, tag="masks3")
        nc.sync.dma_start(out=msk_t, in_=msk_in[:, :])
        m1, m2, m3 = (msk_t[:, 512 * i:512 * (i + 1)] for i in range(3))
        prm = persist.tile([1, 64], bf16, tag="prm")
        nc.vector.memset(prm, 0.0)
        # selector matrices (heads grouped 4 per den tile at partitions
        # 0/32/64/96); sel block A covers even ft, block B odd ft
        sel_t = persist.tile([97, 256], bf16, tag="sel")
        nc.sync.dma_start(out=sel_t, in_=sel_in[:, :])
        # denominator collection tiles: 4 heads each, rows at 0/32/64/96;
        # other partitions memset to 1 so reciprocal stays finite
        den4 = [persist.tile([97, CHUNK], f32, tag=f"den4_{i}") for i in range(4)]
        den4r = [persist.tile([97, CHUNK], bf16, tag=f"den4r{i}") for i in range(4)]
        for i in range(4):
            nc.vector.memset(den4[i], 1.0)

        ln_pool = _patch_tile_name(ctx.enter_context(tc.tile_pool(name="ln", bufs=1)))
        wpool = _patch_tile_name(ctx.enter_context(tc.tile_pool(name="weights", bufs=1)))
        apool = _patch_tile_name(ctx.enter_context(tc.tile_pool(name="acts", bufs=1)))
        kvpool = _patch_tile_name(ctx.enter_context(tc.tile_pool(name="kv", bufs=1)))
        spool = _patch_tile_name(ctx.enter_context(tc.tile_pool(name="small", bufs=2)))

        def layernorm_T(xt, ps_a):
            """LN both token tiles of x -> feature-major bf16 tiles [128, 256] x8."""
            xh = []
            for m in range(2):
                stats = spool.tile([128, 2, 6], f32, tag="lnstats")
                nc.vector.bn_stats(out=stats[:, 0, :], in_=xt[m][:, 0:512])
                nc.vector.bn_stats(out=stats[:, 1, :], in_=xt[m][:, 512:1024])
                mv = spool.tile([128, 2], f32, tag="lnmv")
                nc.vector.bn_aggr(out=mv, in_=stats)
                rs = spool.tile([128, 1], f32, tag="lnrs")
                nc.scalar.activation(out=rs, in_=mv[:, 1:2], func=AF.Sqrt, bias=eps_t)
                nc.vector.reciprocal(out=rs, in_=rs)
                xh_m = ln_pool.tile([128, D], bf16, tag=f"lnxh{m}")
                nc.vector.tensor_scalar(
                    out=xh_m, in0=xt[m], scalar1=mv[:, 0:1], scalar2=rs,
                    op0=mybir.AluOpType.subtract, op1=mybir.AluOpType.mult)
                xh.append(xh_m)
            xhT = [ln_pool.tile([128, CHUNK], bf16, tag=f"lnxhT{t}") for t in range(FT)]
            for t in range(FT):
                for m in range(2):
                    ptr = ps_a.tile([128, 128], bf16, tag="ps256", padded_shape=[128, 512])
                    nc.tensor.transpose(ptr, xh[m][:, 128 * t:128 * (t + 1)], ident)
                    nc.vector.tensor_copy(out=xhT[t][:, 128 * m:128 * (m + 1)], in_=ptr)
            return xhT

        with ExitStack() as lctx:
            ps_a = _patch_tile_name(lctx.enter_context(
                tc.tile_pool(name="ps_a", bufs=2, space="PSUM")))
            ps_b = _patch_tile_name(lctx.enter_context(
                tc.tile_pool(name="ps_b", bufs=2, space="PSUM")))
            ps_att = _patch_tile_name(lctx.enter_context(
                tc.tile_pool(name="ps_att", bufs=2, space="PSUM")))

            # prime the collective stack: tiny AG absorbs the initial barrier
            nc.gpsimd.collective_compute(
                "AllGather", mybir.AluOpType.bypass, replica_groups=groups_b,
                ins=[prime_l[:, :]], outs=[prime_a[:, :]])

            for rep in range(n_rep):
                for l in range(n_layers):
                    _fsid, _ = nc.enter_named_scope(f"L{l}.layer", False)
                    xhT = layernorm_T(x_t, ps_a)
                    k_loc, k_all = k_locs[rep][l], k_alls[rep][l]
                    v_loc, v_all = v_locs[rep][l], v_alls[rep][l]

                    # ---- K^T (feature-major) + ship + AllGather ----
                    wk_t = [wpool.tile([128, D], bf16, tag=f"wk{kk}") for kk in range(FT)]
                    for kk in range(FT):
                        nc.sync.dma_start(
                            out=wk_t[kk],
                            in_=wq_in[l, 128 * kk:128 * (kk + 1), D:2 * D])
                    kTl = [apool.tile([128, CHUNK], bf16, tag=f"kT{t}") for t in range(FT)]
                    for f in range(FT):
                        ps = ps_a.tile([128, CHUNK], f32, tag="ps256")
                        for kk in range(FT):
                            nc.tensor.matmul(ps, wk_t[kk][:, 128 * f:128 * (f + 1)],
                                             xhT[kk], start=(kk == 0), stop=(kk == FT - 1))
                        if f % 2 == 0:
                            nc.vector.tensor_copy(out=kTl[f], in_=ps)
                        else:
                            nc.scalar.activation(out=kTl[f], in_=ps, func=AF.Copy)
                        nc.sync.dma_start(
                            out=dram_ap(k_loc, 128 * f * CHUNK, [[CHUNK, 128], [1, CHUNK]]),
                            in_=kTl[f])
                    nc.gpsimd.collective_compute(
                        "AllGather", mybir.AluOpType.bypass, replica_groups=groups_b,
                        ins=[k_loc[:, :]], outs=[k_all[:, :]])

                    # ---- V (token-major, 65-stride head layout) + ship + AllGather ----
                    wv_t = [wpool.tile([128, D], bf16, tag=f"wv{kk}") for kk in range(FT)]
                    for kk in range(FT):
                        nc.sync.dma_start(
                            out=wv_t[kk],
                            in_=wq_in[l, 128 * kk:128 * (kk + 1), 2 * D:3 * D])
                    v_t = [kvpool.tile([128, 16 * 65], bf16, tag=f"vloc{m}") for m in range(2)]
                    for m in range(2):
                        psA = ps_b.tile([128, 512], f32, tag="ps512")
                        psB = ps_b.tile([128, 512], f32, tag="ps512")
                        for kk in range(FT):
                            lhs = xhT[kk][:, 128 * m:128 * (m + 1)]
                            nc.tensor.matmul(psA, lhs, wv_t[kk][:, 0:512],
                                             start=(kk == 0), stop=(kk == FT - 1))
                            nc.tensor.matmul(psB, lhs, wv_t[kk][:, 512:1024],
                                             start=(kk == 0), stop=(kk == FT - 1))
                        ve = v_t[m].rearrange("p (h c) -> p h c", h=16)
                        nc.vector.tensor_copy(out=ve[:, 0:8, 0:64], in_=psA)
                        nc.vector.tensor_copy(out=ve[:, 8:16, 0:64], in_=psB)
                        nc.vector.memset(ve[:, :, 64:65], 1.0)
                        nc.sync.dma_start(
                            out=dram_ap(v_loc, 128 * m * D, [[D, 128], [1, D]]),
                            in_=ve[:, :, 0:64])
                    nc.gpsimd.collective_compute(
                        "AllGather", mybir.AluOpType.bypass, replica_groups=groups_b,
                        ins=[v_loc[:, :]], outs=[v_all[:, :]])

                    # ---- Q (feature-major) ----
                    wqq_t = [wpool.tile([128, D], bf16, tag=f"wqq{kk}") for kk in range(FT)]
                    for kk in range(FT):
                        nc.sync.dma_start(
                            out=wqq_t[kk],
                            in_=wq_in[l, 128 * kk:128 * (kk + 1), 0:D])
                    qT = [apool.tile([128, CHUNK], bf16, tag=f"qT{t}") for t in range(FT)]
                    for f in range(FT):
                        ps = ps_a.tile([128, CHUNK], f32, tag="ps256")
                        for kk in range(FT):
                            nc.tensor.matmul(ps, wqq_t[kk][:, 128 * f:128 * (f + 1)],
                                             xhT[kk], start=(kk == 0), stop=(kk == FT - 1))
                        if f % 2 == 0:
                            nc.vector.tensor_copy(out=qT[f], in_=ps)
                        else:
                            nc.scalar.activation(out=qT[f], in_=ps, func=AF.Copy)

                    # ---- load gathered interior K tiles (t = 0..6) ----
                    # absolute kv tile t: t<4 -> rank t slot A; t>=4 -> rank 7-t slot B
                    kall = [kvpool.tile([128, 7 * 128], bf16, tag=f"kft{ft}")
                            for ft in range(FT)]
                    kcol = lambda t: 128 * t if t < 4 else 128 * (10 - t)
                    for ft in range(FT):
                        nc.sync.dma_start(
                            out=kall[ft][:, 0:512].rearrange("p (r c) -> p r c", r=4),
                            in_=dram_ap(k_all, 128 * ft * CHUNK,
                                        [[CHUNK, 128], [D * CHUNK, 4], [1, 128]]))
                        nc.sync.dma_start(
                            out=kall[ft][:, 512:896].rearrange("p (r c) -> p r c", r=3),
                            in_=dram_ap(k_all, D * CHUNK + 128 * ft * CHUNK + 128,
                                        [[CHUNK, 128], [D * CHUNK, 3], [1, 128]]))
                    # gathered interior V tiles in 65-stride layout + ones col
                    vext = [kvpool.tile([128, 16 * 65], bf16, tag=f"vext{t}")
                            for t in range(NIT_B)]
                    for t in range(NIT_B):
                        rk, sl = (t, 0) if t < 4 else (7 - t, 1)
                        ve = vext[t].rearrange("p (h c) -> p h c", h=16)
                        nc.sync.dma_start(
                            out=ve[:, :, 0:64],
                            in_=dram_ap(v_all, (rk * CHUNK + 128 * sl) * D,
                                        [[D, 128], [64, 16], [1, 64]]))
                        nc.vector.memset(ve[:, :, 64:65], 1.0)

                    # ---- attention ----
                    attnN = [apool.tile([128, CHUNK], bf16, tag=f"attnN{t}")
                             for t in range(FT)]
                    dbg_e = []
                    for h in range(H):
                        ft, ro = h // 2, 64 * (h % 2)
                        kT_h = lambda t: kall[ft][ro:ro + 64, kcol(t):kcol(t) + 128]
                        qA = qT[ft][ro:ro + 64, 0:128]
                        qB = qT[ft][ro:ro + 64, 128:256]
                        # scores: S1 = [t0 | t1], S2 = [t2 | diagA | diagB], S3 = [t3..t6]
                        s1 = ps_b.tile([128, 512], f32, tag="ps512")
                        nc.tensor.matmul(s1[:, 0:256], kT_h(0), qT[ft][ro:ro + 64, :],
                                         start=True, stop=True)
                        nc.tensor.matmul(s1[:, 256:512], kT_h(1), qT[ft][ro:ro + 64, :],
                                         start=True, stop=True)
                        s2 = ps_b.tile([128, 512], f32, tag="ps512")
                        nc.tensor.matmul(s2[:, 0:256], kT_h(2), qT[ft][ro:ro + 64, :],
                                         start=True, stop=True)
                        nc.tensor.matmul(s2[:, 256:384],
                                         kTl[ft][ro:ro + 64, 0:128], qA,
                                         start=True, stop=True)
                        nc.tensor.matmul(s2[:, 384:512],
                                         kTl[ft][ro:ro + 64, 128:256], qB,
                                         start=True, stop=True)
                        s3 = ps_b.tile([128, 512], f32, tag="ps512")
                        for t in range(3, NIT_B):
                            nc.tensor.matmul(s3[:, 128 * (t - 3):128 * (t - 2)],
                                             kT_h(t), qB, start=True, stop=True)
                        e1 = spool.tile([128, 512], bf16, tag="e1", bufs=4)
                        e2 = spool.tile([128, 512], bf16, tag="e2", bufs=4)
                        e3 = spool.tile([128, 512], bf16, tag="e3", bufs=4)
                        nc.scalar.activation(out=e1, in_=s1, func=AF.Exp, scale=0.125)
                        nc.scalar.activation(out=e2, in_=s2, func=AF.Exp, scale=0.125)
                        nc.scalar.activation(out=e3, in_=s3, func=AF.Exp, scale=0.125)
                        # combined per-core kill + tril masks (one TT per e tile)
                        nc.vector.tensor_mul(out=e1, in0=e1, in1=m1)
                        nc.vector.tensor_mul(out=e2, in0=e2, in1=m2)
                        nc.vector.tensor_mul(out=e3, in0=e3, in1=m3)
                        if dbg and l == 0 and h == 0:
                            dbg_e = [e1, e2]
                        # av accumulation: regions in SEPARATE psum banks —
                        # interleaved accumulating groups sharing a bank break
                        # (a start=True zeroes the other group's partial sums)
                        attA = ps_att.tile([65, 128], f32, tag="attA")
                        attB = ps_att.tile([65, 128], f32, tag="attB")
                        vh = lambda vt: vt[:, 65 * h:65 * h + 65]
                        nc.tensor.matmul(attA, vh(v_t[0]), e2[:, 256:384],
                                         start=True, stop=False)
                        nc.tensor.matmul(attB, vh(v_t[1]), e2[:, 384:512],
                                         start=True, stop=False)
                        rhsA = [e1[:, 0:128], e1[:, 256:384], e2[:, 0:128]]
                        rhsB = [e1[:, 128:256], e1[:, 384:512], e2[:, 128:256]]
                        for t in range(NIT_A):
                            nc.tensor.matmul(attA, vh(vext[t]), rhsA[t],
                                             start=False, stop=(t == NIT_A - 1))
                            nc.tensor.matmul(attB, vh(vext[t]), rhsB[t],
                                             start=False, stop=False)
                        for t in range(3, NIT_B):
                            nc.tensor.matmul(attB, vh(vext[t]),
                                             e3[:, 128 * (t - 3):128 * (t - 2)],
                                             start=False, stop=(t == NIT_B - 1))
                        p0 = 32 * (h % 4)
                        nc.vector.tensor_copy(out=den4[h // 4][p0:p0 + 1, 0:128],
                                              in_=attA[64:65, :])
                        nc.vector.tensor_copy(out=den4[h // 4][p0:p0 + 1, 128:256],
                                              in_=attB[64:65, :])
                        nc.scalar.activation(out=attnN[ft][ro:ro + 64, 0:128],
                                             in_=attA[0:64, :], func=AF.Copy)
                        nc.scalar.activation(out=attnN[ft][ro:ro + 64, 128:256],
                                             in_=attB[0:64, :], func=AF.Copy)

                    for i in range(4):
                        dr = spool.tile([97, CHUNK], f32, tag="denr", bufs=2)
                        nc.vector.reciprocal(out=dr, in_=den4[i])
                        nc.vector.tensor_copy(out=den4r[i], in_=dr)
                    attnT = [apool.tile([128, CHUNK], bf16, tag=f"kT{t}")
                             for t in range(FT)]
                    for ft in range(FT):
                        rb = ps_a.tile([128, CHUNK], f32, tag="ps256")
                        lhs = sel_t[:, 0:128] if ft % 2 == 0 else sel_t[:, 128:256]
                        nc.tensor.matmul(rb, lhs, den4r[ft // 2], start=True, stop=True)
                        nc.vector.tensor_mul(out=attnT[ft], in0=attnN[ft], in1=rb)

                    if dbg and l == 0:
                        def dump(col, ap, rows=128):
                            nc.sync.dma_start(
                                out=dbg_ext[0:rows, col:col + ap.shape[-1]], in_=ap)
                        dump(0, v_t[0][:, 0:1040])        # local V ve layout
                        dump(1040, kTl[0][:, :])          # local K^T ft=0
                        dump(1296, qT[0][:, :])           # Q ft=0
                        dump(1808, attnN[0][:, :])        # numerators ft=0
                        dump(2064, attnT[0][:, :])        # normalized ft=0
                        dump(2320, kall[0][:, :])         # gathered K ft=0
                        dump(3216, dbg_e[0][:, :])        # e1 of head 0
                        dump(3728, dbg_e[1][:, 0:368])    # e2 of head 0 (truncated)
                        nc.sync.dma_start(out=dbgf_ext[0:97, :],
                                          in_=den4[0][:, :])   # denominators heads 0-3

                    # ---- proj + residual (activation-stationary) ----
                    wp_t = [wpool.tile([128, D], bf16, tag=f"wp{kk}") for kk in range(FT)]
                    for kk in range(FT):
                        nc.sync.dma_start(out=wp_t[kk],
                                          in_=wp_in[l, 128 * kk:128 * (kk + 1), :])
                    for m in range(2):
                        psA = ps_b.tile([128, 512], f32, tag="ps512")
                        psB = ps_b.tile([128, 512], f32, tag="ps512")
                        for kk in range(FT):
                            lhs = attnT[kk][:, 128 * m:128 * (m + 1)]
                            nc.tensor.matmul(psA, lhs, wp_t[kk][:, 0:512],
                                             start=(kk == 0), stop=(kk == FT - 1))
                            nc.tensor.matmul(psB, lhs, wp_t[kk][:, 512:1024],
                                             start=(kk == 0), stop=(kk == FT - 1))
                        nc.vector.tensor_add(out=x_t[m][:, 0:512],
                                             in0=x_t[m][:, 0:512], in1=psA)
                        nc.vector.tensor_add(out=x_t[m][:, 512:1024],
                                             in0=x_t[m][:, 512:1024], in1=psB)

                    # ---- MLP ----
                    hT = layernorm_T(x_t, ps_a)
                    w1_t = [wpool.tile([128, D], bf16, tag=f"w1{kk}", bufs=2)
                            for kk in range(FT)]
                    w2_t = [wpool.tile([128, D], bf16, tag=f"w2{kk}")
                            for kk in range(FT)]
                    for kk in range(FT):
                        nc.sync.dma_start(out=w1_t[kk],
                                          in_=w1_in[l, 128 * kk:128 * (kk + 1), :])
                        nc.sync.dma_start(out=w2_t[kk],
                                          in_=w2_in[l, 128 * kk:128 * (kk + 1), :])
                    gT = [apool.tile([128, CHUNK], bf16, tag=f"qT{t}") for t in range(FT)]
                    for f in range(FT):
                        ps = ps_a.tile([128, CHUNK], f32, tag="ps256")
                        for kk in range(FT):
                            nc.tensor.matmul(ps, w1_t[kk][:, 128 * f:128 * (f + 1)],
                                             hT[kk], start=(kk == 0), stop=(kk == FT - 1))
                        nc.scalar.activation(out=gT[f], in_=ps, func=AF.Gelu)
                    for m in range(2):
                        psA = ps_b.tile([128, 512], f32, tag="ps512")
                        psB = ps_b.tile([128, 512], f32, tag="ps512")
                        for kk in range(FT):
                            lhs = gT[kk][:, 128 * m:128 * (m + 1)]
                            nc.tensor.matmul(psA, lhs, w2_t[kk][:, 0:512],
                                             start=(kk == 0), stop=(kk == FT - 1))
                            nc.tensor.matmul(psB, lhs, w2_t[kk][:, 512:1024],
                                             start=(kk == 0), stop=(kk == FT - 1))
                        nc.vector.tensor_add(out=x_t[m][:, 0:512],
                                             in0=x_t[m][:, 0:512], in1=psA)
                        nc.vector.tensor_add(out=x_t[m][:, 512:1024],
                                             in0=x_t[m][:, 512:1024], in1=psB)
                    nc.leave_named_scope(f"L{l}.layer", _fsid, False)

                # ---- final LN + AllGather ----
                _fsid, _ = nc.enter_named_scope("fin.ag", False)
                xfT = layernorm_T(x_t, ps_a)
                xf_loc, xf_all = xf_locs[rep], xf_alls[rep]
                for t in range(FT):
                    nc.sync.dma_start(
                        out=dram_ap(xf_loc, 128 * t * CHUNK, [[CHUNK, 128], [1, CHUNK]]),
                        in_=xfT[t])
                nc.gpsimd.collective_compute(
                    "AllGather", mybir.AluOpType.bypass, replica_groups=group_all,
                    ins=[xf_loc[:, :]], outs=[xf_all[:, :]])
                nc.leave_named_scope("fin.ag", _fsid, False)

        # ---- logits (activation-stationary over vocab chunks) ----
        with ExitStack() as gctx:
            ps_lg = _patch_tile_name(gctx.enter_context(
                tc.tile_pool(name="ps_lg", bufs=4, space="PSUM")))
            _fsid, _ = nc.enter_named_scope("fin.logits", False)
            NCH = [1024, 1024, 1024, 928]
            for rep in range(n_rep):
                xf_all = xf_alls[rep]
                for mh in range(2):   # halves of the 2048 gathered tokens
                    # vocab weights for the first chunk don't depend on the
                    # AllGather — issue them before the AG-blocked xall loads
                    # so they stream during the collective
                    won0 = None
                    if mh == 0:
                        won0 = [wpool.tile([128, 1024], bf16, tag=f"w1{kk}", bufs=2)
                                for kk in range(FT)]
                        for kk in range(FT):
                            nc.sync.dma_start(
                                out=won0[kk][:, :NCH[0]],
                                in_=wo_in[128 * kk:128 * (kk + 1), 0:NCH[0]])
                    xall = [wpool.tile([128, 1024], bf16, tag=f"wk{kk}")
                            for kk in range(FT)]
                    for kk in range(FT):
                        nc.sync.dma_start(
                            out=xall[kk].rearrange("p (r c) -> p r c", r=4),
                            in_=dram_ap(
                                xf_all, (4 * mh) * D * CHUNK + 128 * kk * CHUNK,
                                [[CHUNK, 128], [D * CHUNK, 4], [1, CHUNK]]))
                    for n in range(4):
                        n0 = 1024 * n
                        if n == 0 and won0 is not None:
                            won = won0
                        else:
                            won = [wpool.tile([128, 1024], bf16, tag=f"w1{kk}", bufs=2)
                                   for kk in range(FT)]
                            for kk in range(FT):
                                nc.sync.dma_start(
                                    out=won[kk][:, :NCH[n]],
                                    in_=wo_in[128 * kk:128 * (kk + 1), n0:n0 + NCH[n]])
                        for mm in range(8):
                            ps = ps_lg.tile([128, 1024], f32, tag="lg")
                            for kk in range(FT):
                                lhs = xall[kk][:, 128 * mm:128 * (mm + 1)]
                                nc.tensor.matmul(ps[:, 0:512], lhs, won[kk][:, 0:512],
                                                 start=(kk == 0), stop=(kk == FT - 1))
                                nc.tensor.matmul(ps[:, 512:NCH[n]], lhs,
                                                 won[kk][:, 512:NCH[n]],
                                                 start=(kk == 0), stop=(kk == FT - 1))
                            lg = spool.tile([128, 1024], f32, tag="lgout", bufs=2)
                            if mm % 2 == 0:
                                nc.vector.tensor_copy(out=lg[:, :NCH[n]], in_=ps[:, :NCH[n]])
                            else:
                                nc.scalar.activation(out=lg[:, :NCH[n]], in_=ps[:, :NCH[n]],
                                                     func=AF.Copy)
                            row0 = (8 * mh + mm) * 128
                            nc.sync.dma_start(
                                out=out_ext[row0:row0 + 128, n0:n0 + NCH[n]],
                                in_=lg[:, :NCH[n]])
            nc.leave_named_scope("fin.logits", _fsid, False)

    nc.compile()
    return nc


_CACHE = {}


def _get_program(n_layers=L, n_rep=1, bcast_mode='mm'):
    key = (n_layers, n_rep, bcast_mode)
    if key not in _CACHE:
        _CACHE[key] = build_program(n_layers, n_rep, bcast_mode)
    return _CACHE[key]


_LAST_RESULT = None


def run_model(prep, n_layers=L, n_rep=1, bcast_mode='mm', **run_kwargs):
    global _LAST_RESULT
    from concourse.bass_utils import run_bass_kernel_spmd
    nc = _get_program(n_layers, n_rep, bcast_mode)
    sel = make_sel()
    in_maps = []
    for c in range(N_CORES):
        b, g = c // 4, c % 4
        rows = np.concatenate([
            prep['x0'][b, 128 * g:128 * (g + 1), :],
            prep['x0'][b, 128 * (7 - g):128 * (8 - g), :]])
        in_maps.append({
            'x0': np.ascontiguousarray(rows),
            'wq': prep['w_qkv'][:n_layers],
            'wp': prep['w_proj'][:n_layers],
            'w1': prep['w_fc1'][:n_layers],
            'w2': prep['w_fc2'][:n_layers],
            'wo': np.ascontiguousarray(prep['w_out'][:, VS * c:VS * (c + 1)]),
            'masks3': make_masks3(g),
            'sel': sel,
        })
    res = run_bass_kernel_spmd(nc, in_maps, core_ids=list(range(N_CORES)), **run_kwargs)
    _LAST_RESULT = res
    parts = [res.results[c]['logits'] for c in range(N_CORES)]   # [2048, 4000] each
    wide = np.concatenate(parts, axis=1)                          # [2048, 32000] permuted rows
    # invert the row permutation: rank r holds sub-chunks (g, 7-g) of batch r//4
    full = np.empty((B, T, V), np.float32)
    for r in range(N_CORES):
        b, g = r // 4, r % 4
        full[b, 128 * g:128 * (g + 1)] = wide[256 * r:256 * r + 128]
        full[b, 128 * (7 - g):128 * (8 - g)] = wide[256 * r + 128:256 * (r + 1)]
    return full


def kernel(**inputs):
    prep = host_prep(inputs)
    return run_model(prep)
